# revision 1
# baseline (speedup 1.0000x reference)
"""MaxMargin loss kernel for 8 Trainium2 NeuronCores.

Reference computation (B=8192 rows, D=512, S=25 negative rounds):
    cos_pos[b]   = <y_true[b], y_pred[b]> / max(|y_true[b]||y_pred[b]|, eps)
    cos_neg[s,b] = <y_true[perm[s,b]], y_pred[b]> / max(|y_true[perm[s,b]]||y_pred[b]|, eps)
    out = mean_b( sum_s relu(1 - cos_pos + cos_neg) ) / S

Strategy: data-parallel over the batch dim (1024 rows of y_pred per core).
Each core normalizes the full y_true into a bf16 row table in its DRAM
(one 16MB read + 8MB write), then the permutation "gather" is a DMA row
gather from that table (25 x 1024 rows x 1KB per core).  Normalizing
before the gather folds the n_true[perm] term into the gathered rows, so
no scalar gather of norms is needed.  cos_pos is handled as an extra
identity-index round (round 0).  Dot products run as fused
multiply+row-reduce DVE ops; margins as fused (add, clamp-at-0) ops; the
final cross-partition sum is one 1-column matmul.  Host sums the 8
per-core partials.
"""

import os
import sys

import numpy as np

for _p in ("/opt/trn_rl_repo", "/root/.axon_site/_ro/trn_rl_repo"):
    if os.path.isdir(_p) and _p not in sys.path:
        sys.path.append(_p)

import concourse.bacc as bacc
import concourse.bass as bass
import concourse.mybir as mybir
import concourse.tile as tile
from concourse.bass_utils import run_bass_kernel_spmd

B = 8192          # total batch rows
D = 512           # feature dim
S = 25            # negative-sampling rounds
NCORES = 8
SH = B // NCORES  # rows per core (1024)
NB = SH // 128    # 128-row blocks per core (8)
NSLAB = B // SH   # y_true slabs for the normalize pass (8)
NR = S + 1        # gather rounds incl. identity round 0 (26)
RB = 2            # rounds per dma_gather (2048 idx, multi-packet)
GBUFS = 3         # gather tile triple buffering
NQ = 4            # swdge queues; round-robin gathers across them
F32 = mybir.dt.float32
BF16 = mybir.dt.bfloat16
I16 = mybir.dt.int16

AX = mybir.AxisListType
ALU = mybir.AluOpType
ACTF = mybir.ActivationFunctionType


def build_program(mode="full"):
    """Build the (identical-on-every-core) Bass program.

    mode: "full" | "nogather" (skip gather rounds; cn stays zero) |
          "onegather" (single gather batch) — used for HW bisects.
    """
    nc = bacc.Bacc(None, target_bir_lowering=False, num_swdge_queues=NQ)

    yp = nc.dram_tensor("yp", [SH, D], F32, kind="ExternalInput")
    yt = nc.dram_tensor("yt", [B, D], F32, kind="ExternalInput")
    idx = nc.dram_tensor("idx", [128, NR * 64], I16, kind="ExternalInput")
    tt = nc.dram_tensor("tt", [B, D], BF16, kind="Internal")
    out = nc.dram_tensor("out", [1, 1], F32, kind="ExternalOutput")

    with tile.TileContext(nc) as tc:
        with (
            tc.tile_pool(name="singles", bufs=1) as singles,
            tc.tile_pool(name="slab", bufs=4) as slab_pool,
            tc.tile_pool(name="upool", bufs=4) as u_pool,
            tc.tile_pool(name="gpool", bufs=GBUFS) as g_pool,
            tc.tile_pool(name="scr", bufs=4) as scr_pool,
            tc.tile_pool(name="small", bufs=4) as small_pool,
            tc.tile_pool(name="psum", bufs=1, space="PSUM") as psum_pool,
        ):
            idx_sb = singles.tile([128, NR * 64], I16)
            nc.sync.dma_start(out=idx_sb, in_=idx[:, :])

            zero_b = singles.tile([128, 1], F32)
            nc.vector.memset(zero_b, 0.0)

            def fused_dot(in0, in1, accum_out):
                """accum_out[:, 0] = rowsum(in0 * in1); one DVE pass."""
                scr = scr_pool.tile([128, D], F32, tag="dot_scr")
                nc.vector.scalar_tensor_tensor(
                    out=scr,
                    in0=in0,
                    scalar=1.0,
                    in1=in1,
                    op0=ALU.mult,
                    op1=ALU.mult,
                    accum_out=accum_out,
                )

            def rowwise_rsqrt_ssq(x):
                """x: [128, NB, D] f32 -> per-row 1/sqrt(sum(x^2)) [128, NB].

                Squares alternate DVE/ACT to split the load; the chain runs
                max+recip on DVE then sqrt on ACT (rs = sqrt(1/ssq)) so each
                slab crosses engines once instead of three times.
                """
                ssq = small_pool.tile([128, NB], F32, tag="ssq")
                for n in range(NB):
                    if n % 2 == 0:
                        fused_dot(x[:, n, :], x[:, n, :], ssq[:, n : n + 1])
                    else:
                        act_scr = scr_pool.tile([128, D], F32, tag="act_scr")
                        nc.scalar.activation(
                            out=act_scr,
                            in_=x[:, n, :],
                            func=ACTF.Square,
                            bias=0.0,
                            scale=1.0,
                            accum_out=ssq[:, n : n + 1],
                        )
                ssqm = small_pool.tile([128, NB], F32, tag="ssqm")
                nc.vector.tensor_scalar_max(out=ssqm, in0=ssq, scalar1=1e-30)
                inv = small_pool.tile([128, NB], F32, tag="inv")
                nc.vector.reciprocal(out=inv, in_=ssqm)
                rs = small_pool.tile([128, NB], F32, tag="rs")
                nc.scalar.activation(
                    out=rs, in_=inv, func=ACTF.Sqrt, bias=zero_b, scale=1.0
                )
                return rs

            # ---- y_pred shard: load + normalize (kept f32 in SBUF) ----
            up = singles.tile([128, NB, D], F32)
            xp = slab_pool.tile([128, NB, D], F32, tag="x")
            nc.sync.dma_start(
                out=xp, in_=yp[:, :].rearrange("(n p) d -> p n d", p=128)
            )
            rs_p = rowwise_rsqrt_ssq(xp)
            for n in range(NB):
                nc.scalar.activation(
                    out=up[:, n, :],
                    in_=xp[:, n, :],
                    func=ACTF.Copy,
                    bias=0.0,
                    scale=rs_p[:, n : n + 1],
                )

            # ---- normalize full y_true into bf16 DRAM table ----
            for l in range(NSLAB):
                x = slab_pool.tile([128, NB, D], F32, tag="x")
                nc.sync.dma_start(
                    out=x,
                    in_=yt[l * SH : (l + 1) * SH, :].rearrange(
                        "(n p) d -> p n d", p=128
                    ),
                )
                rs_t = rowwise_rsqrt_ssq(x)
                u = u_pool.tile([128, NB, D], BF16, tag="u")
                for n in range(NB):
                    nc.scalar.activation(
                        out=u[:, n, :],
                        in_=x[:, n, :],
                        func=ACTF.Copy,
                        bias=0.0,
                        scale=rs_t[:, n : n + 1],
                    )
                nc.sync.dma_start(
                    out=tt[l * SH : (l + 1) * SH, :].rearrange(
                        "(n p) d -> p n d", p=128
                    ),
                    in_=u,
                )

            # ---- gather rounds + fused dot products ----
            # CN[:, n, r] = cos of round r for row block n (round 0 = cos_pos)
            cn = singles.tile([128, NB, NR], F32)
            nc.vector.memset(cn, 0.0)
            starts = list(range(0, NR, RB))
            starts = {"full": starts, "nogather": [], "onegather": starts[:1]}[mode]
            for bi, s0 in enumerate(starts):
                rb = min(RB, NR - s0)  # tail batch may be short
                g = g_pool.tile([128, RB * NB, D], BF16, tag="g")
                nc.gpsimd.dma_gather(
                    g[:, : rb * NB, :],
                    tt[:, :],
                    idx_sb[:, s0 * 64 : (s0 + rb) * 64],
                    num_idxs=rb * SH,
                    num_idxs_reg=rb * SH,
                    elem_size=D,
                    # >64 descriptors per SDMA channel needs multi-packet
                    single_packet=(rb * SH // 16) <= 64,
                    queue_num=bi % NQ,
                )
                for r in range(rb):
                    s = s0 + r
                    for n in range(NB):
                        fused_dot(
                            g[:, r * NB + n, :],
                            up[:, n, :],
                            cn[:, n, s : s + 1],
                        )

            # ---- margins: sum_s relu((1 - cos_pos) + cos_neg) ----
            cpb = singles.tile([128, NB], F32)  # 1 - cos_pos
            nc.vector.tensor_scalar(
                out=cpb,
                in0=cn[:, :, 0],
                scalar1=-1.0,
                scalar2=1.0,
                op0=ALU.mult,
                op1=ALU.add,
            )
            mt = singles.tile([128, NB], F32)
            for n in range(NB):
                m_scr = scr_pool.tile([128, S], F32, tag="m_scr")
                nc.vector.tensor_scalar(
                    out=m_scr,
                    in0=cn[:, n, 1:NR],
                    scalar1=cpb[:, n : n + 1],
                    scalar2=0.0,
                    op0=ALU.add,
                    op1=ALU.max,
                )
                nc.vector.reduce_sum(
                    out=mt[:, n : n + 1], in_=m_scr, axis=AX.X
                )

            # ---- partial = sum over partitions and blocks ----
            mts = singles.tile([128, 1], F32)
            nc.vector.reduce_sum(out=mts, in_=mt, axis=AX.X)
            ones = singles.tile([128, 1], F32)
            nc.vector.memset(ones, 1.0)
            ps = psum_pool.tile([1, 1], F32)
            nc.tensor.matmul(ps, ones, mts, start=True, stop=True)
            osb = singles.tile([1, 1], F32)
            nc.vector.tensor_copy(out=osb, in_=ps)
            nc.sync.dma_start(out=out[:, :], in_=osb)

    return nc


def make_in_maps(y_pred, y_true, perm):
    """Shard the full inputs into the 8 per-core input maps."""
    y_pred = np.ascontiguousarray(y_pred, dtype=np.float32)
    y_true = np.ascontiguousarray(y_true, dtype=np.float32)
    perm = np.asarray(perm)
    in_maps = []
    for c in range(NCORES):
        ident = (c * SH + np.arange(SH, dtype=np.int64))[None, :]
        rounds = np.concatenate(
            [ident, perm[:, c * SH : (c + 1) * SH].astype(np.int64)], axis=0
        )  # [NR, SH]
        # dma_gather index layout: flat index i lives at partition i%16,
        # free slot i//16, replicated across the 8 groups of 16 partitions.
        w = rounds.reshape(NR, SH // 16, 16).transpose(0, 2, 1)  # [NR,16,64]
        rep = np.broadcast_to(w[:, None, :, :], (NR, 8, 16, SH // 16))
        idx = (
            rep.reshape(NR, 128, SH // 16)
            .transpose(1, 0, 2)
            .reshape(128, NR * (SH // 16))
            .astype(np.int16)
        )
        in_maps.append(
            {
                "yp": np.ascontiguousarray(y_pred[c * SH : (c + 1) * SH]),
                "yt": y_true,
                "idx": np.ascontiguousarray(idx),
            }
        )
    return in_maps


_prog_cache = {}


def _get_program():
    if "nc" not in _prog_cache:
        nc = build_program()
        if not nc.is_finalized():
            nc.finalize()  # run Bacc passes (reg alloc, library loads)
        _prog_cache["nc"] = nc
    return _prog_cache["nc"]


def kernel(y_pred, y_true, perm, **run_kwargs):
    nc = _get_program()
    in_maps = make_in_maps(y_pred, y_true, perm)
    res = run_bass_kernel_spmd(
        nc, in_maps, core_ids=list(range(NCORES)), **run_kwargs
    )
    total = sum(float(r["out"][0, 0]) for r in res.results)
    out = np.float32(total / (B * S))
    if run_kwargs:
        return out, res
    return out

